# revision 1
# baseline (speedup 1.0000x reference)
"""AxialSelfAttention Trainium2 Bass kernel.

Reference computation (per batch b):
    xs  = x[b] reshaped [N=2048, E=512]
    qkv = xs @ W + bias                      # [N, 3E]
    q, k, v = split(qkv)
    row:  P = softmax(q @ k.T / sqrt(E));  out_row = P @ v
    col:  A = softmax(q.T @ k / sqrt(E));  out_col = v @ A.T
    out = out_row + out_col                  # [N, E]

Sharding: data-parallel over batch B=32 across 8 cores (4 batches/core).

Per-core kernel layout strategy:
  - x[b] loaded naturally [n,e], transposed on the PE to xT [e-part, n].
  - All six projection layouts are computed by fp32r matmuls from xT:
      Qt, Kt  [e-part, n]  fp32  (stationary/moving operands of row S)
      Q, K    [n-part, e]  bf16  (streamed; operands of col S, accumulated
                                  in PSUM across the 16 token chunks)
      V       [n-part, e]  bf16  (moving operand of row PV)
      Vt      [e-part, n]  bf16  (stationary operand of col output)
  - Softmax skips the max-subtraction (logits are O(+-10) here, exp is safe
    in fp32) so exp() + accum_out gives the row sums in one ACT pass.
  - P~ = exp(S) is normalized by 1/rowsum on DVE, then transposed bf16 via
    the DMA XBAR, so the PV matmul needs no PE transposes or extra PSUM.
  - PV (16 matmuls) and the col-attention output (4 matmuls) accumulate
    into the same PSUM tile, giving out_row + out_col directly.
"""

import sys

for _p in ("/opt/trn_rl_repo", "/root/.axon_site/_ro/trn_rl_repo"):
    if _p not in sys.path:
        sys.path.append(_p)

import numpy as np

B, N, E = 32, 2048, 512
NCORES = 8
NB = B // NCORES  # batches per core
NE = N // 128  # 16 token chunks
ED = E // 128  # 4 feature chunks
SCALE = 1.0 / float(np.sqrt(E))

_NC_CACHE = {}


def build_nc(nb=NB, variant=13, reps=1):
    """Build (once) the single-core Bass module processing nb batches.

    variant 1: all six projection layouts via fp32r matmuls from xT.
    variant 2: like 1, but Vt comes from a bf16 DMA(XBAR)-transpose of V
               instead of its own matmul projection (-64 matmuls/batch).
    variant 3: like 2, and Qt/Kt also come from DMA-transposes of the bf16
               natural q/k (row-attention S then runs in bf16;
               -128 more matmuls/batch).
    variant 4: like 3, plus V/Acol/AcolT double-buffered across batches so
               batch b+1's projection phase (PE) can stream while batch b's
               row attention still reads V.
    variant 5: variant 2 + early transposes: exp quarters are transposed
               unnormalized as soon as they exist, and the 1/rowsum lands on
               the PV output (per-partition ACT scale) instead of on P~;
               PV and the col output use separate PSUM tiles.
    variant 6: variant 5 + the double-buffering of variant 4.
    variant 7: variant 6 + 3-deep prow pool.
    """
    FEAT = {
        1: set(),
        2: {"vt_dma"},
        3: {"vt_dma", "qkt_dma"},
        4: {"vt_dma", "qkt_dma", "dbuf"},
        5: {"vt_dma", "early_t"},
        6: {"vt_dma", "early_t", "dbuf"},
        7: {"vt_dma", "early_t", "dbuf", "prow3"},
        8: {"vt_dma", "spread"},
        9: {"vt_dma", "early_t", "spread"},
        10: {"vt_dma", "late_norm", "act_drain"},
        11: {"vt_dma", "act_drain"},
        13: {"vt_dma", "late_norm", "pe_pt", "dve_merge"},
        14: {"vt_dma", "late_norm", "dve_merge"},
        # timing-only diagnostics (wrong results): fake / absent pt transposes
        15: {"vt_dma", "fake_t"},
        16: {"vt_dma", "no_t"},
        17: {"late_norm", "pe_pt", "dve_merge"},
        18: {"vt_dma", "late_norm", "pe_pt", "dve_merge", "prow3"},
        20: {"vt_dma", "late_norm", "pe_pt", "dve_merge", "psum2"},
        22: {"late_norm", "pe_pt", "dve_merge", "f32r_pv", "slice_xt"},
        23: {"late_norm", "pe_pt", "dve_merge", "f32r_pv", "slice_xt", "prow3"},
    }
    feat = FEAT[variant]
    if (nb, variant, reps) in _NC_CACHE:
        return _NC_CACHE[(nb, variant, reps)]

    import concourse.bass as bass
    import concourse.tile as tile
    from concourse import bacc, mybir
    from concourse.masks import make_identity

    f32 = mybir.dt.float32
    f32r = mybir.dt.float32r
    bf16 = mybir.dt.bfloat16
    AF = mybir.ActivationFunctionType
    AX = mybir.AxisListType

    nc = bacc.Bacc("TRN2")
    x = nc.dram_tensor("x", [nb, N, E], f32, kind="ExternalInput")
    w = nc.dram_tensor("w", [E, 3 * E], f32, kind="ExternalInput")
    bvec = nc.dram_tensor("b", [3 * E], f32, kind="ExternalInput")
    y = nc.dram_tensor("y", [nb, N, E], f32, kind="ExternalOutput")

    with tile.TileContext(nc) as tc:
        with (
            tc.tile_pool(name="const", bufs=1) as constp,
            tc.tile_pool(name="xn", bufs=2 if ("prow3" in feat or "slice_xt" in feat) else 3) as xnp,
            tc.tile_pool(name="big", bufs=1) as bigp,
            tc.tile_pool(name="xsl", bufs=2) as xslp,
            tc.tile_pool(name="qkn", bufs=3) as qknp,
            tc.tile_pool(name="prow", bufs=3 if "prow3" in feat else 2) as prowp,
            tc.tile_pool(name="stat", bufs=3) as statp,
            tc.tile_pool(name="outp", bufs=2) as outpp,
            tc.tile_pool(name="ps_proj", bufs=2, space="PSUM") as ps_proj,
            tc.tile_pool(name="ps_sc", bufs=4, space="PSUM") as ps_sc,
            tc.tile_pool(name="ps_s", bufs=2, space="PSUM") as ps_s,
        ):
            # ---------------- constants ----------------
            # W lands as float32r (rounded by the DVE copy) so fp32r matmuls
            # accept it; staged through the small xn pool to save SBUF.
            W = constp.tile([128, ED, 3 * E], f32r)
            wv = w[:].rearrange("(k p) m -> p k m", p=128)
            for k in range(ED):
                for c in range(3):
                    wst = xnp.tile([128, E], f32, tag="xn", name=f"wst{k}_{c}")
                    nc.sync.dma_start(wst, wv[:, k, c * E : (c + 1) * E])
                    nc.vector.tensor_copy(W[:, k, c * E : (c + 1) * E], wst)

            # bias broadcast across partitions (for [n-part, e] layouts)
            b3 = bvec[:].rearrange("(c m) -> c m", m=E)
            bb = constp.tile([128, 3, E], bf16)
            nc.gpsimd.dma_start(
                bb, bass.AP(tensor=b3.tensor, offset=b3.offset, ap=[[0, 128]] + list(b3.ap))
            )
            # bias per partition (for [e-part, n] layouts): bpart[p, c] = b[c*128+p]
            bpart = constp.tile([128, 3 * ED], f32)
            nc.gpsimd.dma_start(bpart, bvec[:].rearrange("(c p) -> p c", p=128))

            ident = constp.tile([128, 128], f32)
            make_identity(nc, ident)
            identB = constp.tile([128, 128], bf16)
            make_identity(nc, identB)

            def batch_body():
              for b in range(nb):
                qkt_dt = bf16 if "qkt_dma" in feat else f32r
                vn_dt = f32r if "f32r_pv" in feat else bf16
                dbufs = 2 if "dbuf" in feat else 1
                slice_xt = "slice_xt" in feat
                if not slice_xt:
                    xT = bigp.tile([128, ED, N], f32r, tag="xT")
                Qt = bigp.tile([128, ED, N], qkt_dt, tag="Qt")
                Kt = bigp.tile([128, ED, N], qkt_dt, tag="Kt")
                Vn = bigp.tile([128, NE, E], vn_dt, tag="Vn", bufs=dbufs)
                Vt = bigp.tile([128, ED, N], bf16, tag="Vt")
                Acol = bigp.tile([128, ED, E], bf16, tag="Acol", bufs=dbufs)
                AcolT = bigp.tile([128, ED, E], bf16, tag="AcolT", bufs=dbufs)
                scol_ps = [
                    ps_sc.tile([128, E], f32, tag="scps", name=f"scol{b}_{i}")
                    for i in range(ED)
                ]

                # ---- phase A: load x, build xT, projections, col-S accumulation
                for j in range(NE):
                    s_idx, jj = j // ED, j % ED
                    if slice_xt:
                        if jj == 0:
                            xT = xslp.tile(
                                [128, ED, ED * 128], f32r, tag="xsl", name=f"xsl{b}_{s_idx}"
                            )
                        xoff, roff = jj * 128, 0
                    else:
                        xoff, roff = j * 128, s_idx * 512
                    xn = xnp.tile([128, E], f32, tag="xn")
                    nc.sync.dma_start(xn, x[b, j * 128 : (j + 1) * 128, :])
                    for k in range(ED):
                        tps = ps_proj.tile([128, 128], f32, tag="ps")
                        nc.tensor.transpose(tps, xn[:, k * 128 : (k + 1) * 128], ident)
                        nc.vector.tensor_copy(xT[:, k, xoff : xoff + 128], tps)

                    # natural-layout q, k, v for this token chunk
                    qn = qknp.tile([128, E], bf16, tag="qn")
                    kn = qknp.tile([128, E], bf16, tag="kn")
                    for dst, ci in ((qn, 0), (kn, 1), (Vn[:, j, :], 2)):
                        pp = ps_proj.tile([128, E], f32, tag="ps")
                        for k in range(ED):
                            nc.tensor.matmul(
                                pp,
                                xT[:, k, xoff : xoff + 128],
                                W[:, k, ci * E : (ci + 1) * E],
                                start=(k == 0),
                                stop=(k == ED - 1),
                            )
                        nc.vector.tensor_add(dst, pp, bb[:, ci, :])

                    # col-attention S accumulation: S_col[d,e] += q_j.T @ k_j
                    for i in range(ED):
                        nc.tensor.matmul(
                            scol_ps[i],
                            qn[:, i * 128 : (i + 1) * 128],
                            kn,
                            start=(j == 0),
                            stop=(j == NE - 1),
                        )

                    # bf16 transposed layouts via the DMA XBAR (free wrt PE)
                    jsl = slice(j * 128, (j + 1) * 128)
                    if "vt_dma" in feat:
                        nc.scalar.dma_start_transpose(Vt[:, :, jsl], Vn[:, j, :])
                    if "qkt_dma" in feat:
                        nc.scalar.dma_start_transpose(Qt[:, :, jsl], qn)
                        nc.scalar.dma_start_transpose(Kt[:, :, jsl], kn)

                    # transposed-layout projections, one 512-token slice at a time
                    if "qkt_dma" in feat:
                        tproj = ()
                    elif "vt_dma" in feat:
                        tproj = ((Qt, 0), (Kt, 1))
                    else:
                        tproj = ((Qt, 0), (Kt, 1), (Vt, 2))
                    if j % ED == ED - 1 and tproj:
                        sl = slice(s_idx * 512, (s_idx + 1) * 512)
                        for dst, ci in tproj:
                            for i in range(ED):
                                pp = ps_proj.tile([128, E], f32, tag="ps")
                                for k in range(ED):
                                    nc.tensor.matmul(
                                        pp,
                                        W[:, k, ci * E + i * 128 : ci * E + (i + 1) * 128],
                                        xT[:, k, roff : roff + 512],
                                        start=(k == 0),
                                        stop=(k == ED - 1),
                                    )
                                if "act_drain" in feat:
                                    nc.scalar.activation(
                                        out=dst[:, i, sl],
                                        in_=pp,
                                        func=AF.Identity,
                                        bias=bpart[:, ci * ED + i : ci * ED + i + 1],
                                    )
                                else:
                                    nc.vector.tensor_scalar_add(
                                        dst[:, i, sl], pp, bpart[:, ci * ED + i : ci * ED + i + 1]
                                    )

                # ---- phase A2: col softmax + transpose of A
                # col logits are O(+-600): subtract the per-row max (as an ACT
                # bias of -max*SCALE) before exp, unlike the row path.
                cstat = statp.tile([128, 3 * ED], f32, tag="cstat")
                for i in range(ED):
                    nm = cstat[:, 2 * ED + i : 2 * ED + i + 1]
                    nc.vector.reduce_max(nm, scol_ps[i], axis=AX.X, negate=True)
                    nc.vector.tensor_scalar_mul(nm, nm, SCALE)
                    nc.scalar.activation(
                        out=Acol[:, i, :],
                        in_=scol_ps[i],
                        func=AF.Exp,
                        scale=SCALE,
                        bias=nm,
                        accum_out=cstat[:, i : i + 1],
                    )
                nc.vector.reciprocal(cstat[:, ED : 2 * ED], cstat[:, 0:ED])
                for i in range(ED):
                    nc.vector.tensor_scalar_mul(
                        Acol[:, i, :], Acol[:, i, :], cstat[:, ED + i : ED + i + 1]
                    )
                    nc.scalar.dma_start_transpose(
                        AcolT[:, :, i * 128 : (i + 1) * 128], Acol[:, i, :]
                    )

                # ---- phase B: row attention + merged output, per token chunk
                early_t = "early_t" in feat
                late_norm = "late_norm" in feat
                spread = "spread" in feat
                for j in range(NE):
                    teng = (nc.sync if j % 2 else nc.scalar) if spread else nc.scalar
                    yeng = (nc.scalar if j % 2 else nc.sync) if spread else nc.sync
                    pt = prowp.tile([128, N], bf16, tag="pt")
                    ptT = prowp.tile(
                        [128, NE, 128], f32r if "f32r_pv" in feat else bf16, tag="ptT"
                    )
                    rstat = statp.tile([128, 8], f32, tag="rstat")
                    for q in range(4):
                        sps = ps_s.tile([128, 512], f32, tag="s")
                        for k in range(ED):
                            nc.tensor.matmul(
                                sps,
                                Qt[:, k, j * 128 : (j + 1) * 128],
                                Kt[:, k, q * 512 : (q + 1) * 512],
                                start=(k == 0),
                                stop=(k == ED - 1),
                            )
                        nc.scalar.activation(
                            out=pt[:, q * 512 : (q + 1) * 512],
                            in_=sps,
                            func=AF.Exp,
                            scale=SCALE,
                            accum_out=rstat[:, q : q + 1],
                        )
                        if early_t:
                            # transpose the unnormalized quarter right away;
                            # 1/rowsum is applied to the PV output instead
                            teng.dma_start_transpose(
                                ptT[:, 4 * q : 4 * q + 4, :],
                                pt[:, q * 512 : (q + 1) * 512],
                            )
                        if "pe_pt" in feat:
                            for t in range(4):
                                m = 4 * q + t
                                psB = ps_proj.tile(
                                    [128, 128], bf16, tag="ps", name=f"psB{b}_{j}_{m}"
                                )
                                nc.tensor.transpose(
                                    psB, pt[:, m * 128 : (m + 1) * 128], identB
                                )
                                nc.vector.tensor_copy(ptT[:, m, :], psB)
                    nc.vector.reduce_sum(rstat[:, 4:5], rstat[:, 0:4], axis=AX.X)
                    nc.vector.reciprocal(rstat[:, 5:6], rstat[:, 4:5])
                    if late_norm:
                        if "pe_pt" not in feat:
                            teng.dma_start_transpose(ptT, pt)
                    elif not early_t:
                        nc.vector.tensor_scalar_mul(pt, pt, rstat[:, 5:6])
                        if "fake_t" in feat:
                            teng.dma_start(ptT.rearrange("p a b -> p (a b)"), pt)
                        elif "no_t" in feat:
                            nc.vector.tensor_copy(ptT[:, 0, :], pt[:, :128])
                        else:
                            teng.dma_start_transpose(ptT, pt)

                    po = ps_sc.tile([128, E], f32, tag="scps")
                    for m in range(NE):
                        nc.tensor.matmul(
                            po,
                            ptT[:, m, :],
                            Vn[:, m, :],
                            start=(m == 0),
                            stop=((early_t or late_norm) and m == NE - 1),
                        )
                    ot = outpp.tile([128, E], f32, tag="ot")
                    if early_t or late_norm:
                        oc = ps_sc.tile([128, E], f32, tag="scps")
                        for c in range(ED):
                            nc.tensor.matmul(
                                oc,
                                Vt[:, c, j * 128 : (j + 1) * 128],
                                AcolT[:, c, :],
                                start=(c == 0),
                                stop=(c == ED - 1),
                            )
                        if "dve_merge" in feat:
                            if "psum2" in feat:
                                nc.vector.scalar_tensor_tensor(
                                    ot,
                                    po,
                                    rstat[:, 5:6],
                                    oc,
                                    op0=mybir.AluOpType.mult,
                                    op1=mybir.AluOpType.add,
                                )
                            else:
                                octmp = outpp.tile([128, E], f32, tag="octmp")
                                nc.vector.tensor_copy(octmp, oc)
                                nc.vector.scalar_tensor_tensor(
                                    ot,
                                    po,
                                    rstat[:, 5:6],
                                    octmp,
                                    op0=mybir.AluOpType.mult,
                                    op1=mybir.AluOpType.add,
                                )
                        else:
                            nc.scalar.activation(
                                out=ot, in_=po, func=AF.Copy, scale=rstat[:, 5:6]
                            )
                            nc.vector.tensor_add(ot, ot, oc)
                    else:
                        for c in range(ED):
                            nc.tensor.matmul(
                                po,
                                Vt[:, c, j * 128 : (j + 1) * 128],
                                AcolT[:, c, :],
                                start=False,
                                stop=(c == ED - 1),
                            )
                        nc.vector.tensor_copy(ot, po)
                    yeng.dma_start(y[b, j * 128 : (j + 1) * 128, :], ot)

            if reps == 1:
                batch_body()
            else:
                with tc.For_i(0, reps, 1):
                    batch_body()

    nc.compile()
    _NC_CACHE[(nb, variant, reps)] = nc
    return nc


def make_in_maps(x, w_qkv, b_qkv):
    xs = np.ascontiguousarray(np.asarray(x, dtype=np.float32)).reshape(B, N, E)
    w = np.ascontiguousarray(np.asarray(w_qkv, dtype=np.float32))
    bq = np.ascontiguousarray(np.asarray(b_qkv, dtype=np.float32))
    return [
        {"x": np.ascontiguousarray(xs[c * NB : (c + 1) * NB]), "w": w, "b": bq}
        for c in range(NCORES)
    ]


BEST_VARIANT = 13


def kernel(x, w_qkv, b_qkv):
    from concourse.bass_utils import run_bass_kernel_spmd

    nc = build_nc(NB, BEST_VARIANT)
    in_maps = make_in_maps(x, w_qkv, b_qkv)
    res = run_bass_kernel_spmd(nc, in_maps, core_ids=list(range(NCORES)))
    out = np.empty((B, N, E), dtype=np.float32)
    for c in range(NCORES):
        out[c * NB : (c + 1) * NB] = res.results[c]["y"]
    return out



# revision 39
# speedup vs baseline: 37.6654x; 37.6654x over previous
"""AxialSelfAttention Trainium2 Bass kernel.

Reference computation (per batch b):
    xs  = x[b] reshaped [N=2048, E=512]
    qkv = xs @ W + bias                      # [N, 3E]
    q, k, v = split(qkv)
    row:  P = softmax(q @ k.T / sqrt(E));  out_row = P @ v
    col:  A = softmax(q.T @ k / sqrt(E));  out_col = v @ A.T
    out = out_row + out_col                  # [N, E]

Sharding: data-parallel over batch B=32 across 8 cores (4 batches/core).

Best variant (37) per-core layout strategy:
  - Phase A, per 128-token chunk: x loaded naturally, PE-transposed one
    chunk ahead (pipelined) into per-chunk xT slices (f32r); natural
    q/k/v projections from xT (f32r matmuls, full precision for the
    col path); col-attention S accumulated in 4 persistent PSUM banks
    across all 16 chunks.
  - Qt/Kt (row-S operands, [e-part, n] bf16) come from DMA-XBAR
    transposes of bf16 copies of q/k -- zero PE cost, latency hidden
    because row-S only starts once 4 chunks are available.  Vt
    likewise.  The ones columns inside the padded V tile (positions 0
    and 257) make the PV matmul emit the softmax row sums directly.
  - Phase B1 computes row-attention S TRANSPOSED (S.T[m, n], m on
    partitions): exp(S.T) chunks land in SBUF as PT and feed the PV
    matmul as stationary operands directly -- no P transposes at all.
  - Phase B2, per token chunk: PV accumulates [rowsum | out_row] via
    two 257-wide PSUM chains; the col output accumulates from Vt @
    AcolT; one DVE scalar_tensor_tensor merges out_row/rowsum +
    out_col.
  - The col softmax (A2) runs at high scheduler priority so its PSUM
    banks free early for B2; softmax max-subtraction is skipped on the
    row path (logits are O(+-10)) but kept on the col path (O(+-600)).
Measured on HW (8 cores, marginal time per kernel invocation via
hardware-loop reps): ~1.22 ms vs 1.44-1.56 ms for the previous best
(variant 13) in the same session.
"""

import sys

for _p in ("/opt/trn_rl_repo", "/root/.axon_site/_ro/trn_rl_repo"):
    if _p not in sys.path:
        sys.path.append(_p)

import numpy as np

B, N, E = 32, 2048, 512
NCORES = 8
NB = B // NCORES  # batches per core
NE = N // 128  # 16 token chunks
ED = E // 128  # 4 feature chunks
SCALE = 1.0 / float(np.sqrt(E))

_NC_CACHE = {}


def build_nc(nb=NB, variant=13, reps=1):
    """Build (once) the single-core Bass module processing nb batches.

    variant 1: all six projection layouts via fp32r matmuls from xT.
    variant 2: like 1, but Vt comes from a bf16 DMA(XBAR)-transpose of V
               instead of its own matmul projection (-64 matmuls/batch).
    variant 3: like 2, and Qt/Kt also come from DMA-transposes of the bf16
               natural q/k (row-attention S then runs in bf16;
               -128 more matmuls/batch).
    variant 4: like 3, plus V/Acol/AcolT double-buffered across batches so
               batch b+1's projection phase (PE) can stream while batch b's
               row attention still reads V.
    variant 5: variant 2 + early transposes: exp quarters are transposed
               unnormalized as soon as they exist, and the 1/rowsum lands on
               the PV output (per-partition ACT scale) instead of on P~;
               PV and the col output use separate PSUM tiles.
    variant 6: variant 5 + the double-buffering of variant 4.
    variant 7: variant 6 + 3-deep prow pool.
    """
    FEAT = {
        1: set(),
        2: {"vt_dma"},
        3: {"vt_dma", "qkt_dma"},
        4: {"vt_dma", "qkt_dma", "dbuf"},
        5: {"vt_dma", "early_t"},
        6: {"vt_dma", "early_t", "dbuf"},
        7: {"vt_dma", "early_t", "dbuf", "prow3"},
        8: {"vt_dma", "spread"},
        9: {"vt_dma", "early_t", "spread"},
        10: {"vt_dma", "late_norm", "act_drain"},
        11: {"vt_dma", "act_drain"},
        13: {"vt_dma", "late_norm", "pe_pt", "dve_merge"},
        14: {"vt_dma", "late_norm", "dve_merge"},
        # timing-only diagnostics (wrong results): fake / absent pt transposes
        15: {"vt_dma", "fake_t"},
        16: {"vt_dma", "no_t"},
        17: {"late_norm", "pe_pt", "dve_merge"},
        18: {"vt_dma", "late_norm", "pe_pt", "dve_merge", "prow3"},
        20: {"vt_dma", "late_norm", "pe_pt", "dve_merge", "psum2"},
        22: {"late_norm", "pe_pt", "dve_merge", "f32r_pv", "slice_xt"},
        23: {"late_norm", "pe_pt", "dve_merge", "f32r_pv", "slice_xt", "prow3"},
    }
    if variant >= 30:
        return build_nc_v30(nb, variant, reps)
    feat = FEAT[variant]
    if (nb, variant, reps) in _NC_CACHE:
        return _NC_CACHE[(nb, variant, reps)]

    import concourse.bass as bass
    import concourse.tile as tile
    from concourse import bacc, mybir
    from concourse.masks import make_identity

    f32 = mybir.dt.float32
    f32r = mybir.dt.float32r
    bf16 = mybir.dt.bfloat16
    AF = mybir.ActivationFunctionType
    AX = mybir.AxisListType

    nc = bacc.Bacc("TRN2")
    x = nc.dram_tensor("x", [nb, N, E], f32, kind="ExternalInput")
    w = nc.dram_tensor("w", [E, 3 * E], f32, kind="ExternalInput")
    bvec = nc.dram_tensor("b", [3 * E], f32, kind="ExternalInput")
    y = nc.dram_tensor("y", [nb, N, E], f32, kind="ExternalOutput")

    with tile.TileContext(nc) as tc:
        with (
            tc.tile_pool(name="const", bufs=1) as constp,
            tc.tile_pool(name="xn", bufs=2 if ("prow3" in feat or "slice_xt" in feat) else 3) as xnp,
            tc.tile_pool(name="big", bufs=1) as bigp,
            tc.tile_pool(name="xsl", bufs=2) as xslp,
            tc.tile_pool(name="qkn", bufs=3) as qknp,
            tc.tile_pool(name="prow", bufs=3 if "prow3" in feat else 2) as prowp,
            tc.tile_pool(name="stat", bufs=3) as statp,
            tc.tile_pool(name="outp", bufs=2) as outpp,
            tc.tile_pool(name="ps_proj", bufs=2, space="PSUM") as ps_proj,
            tc.tile_pool(name="ps_sc", bufs=4, space="PSUM") as ps_sc,
            tc.tile_pool(name="ps_s", bufs=2, space="PSUM") as ps_s,
        ):
            # ---------------- constants ----------------
            # W lands as float32r (rounded by the DVE copy) so fp32r matmuls
            # accept it; staged through the small xn pool to save SBUF.
            W = constp.tile([128, ED, 3 * E], f32r)
            wv = w[:].rearrange("(k p) m -> p k m", p=128)
            for k in range(ED):
                for c in range(3):
                    wst = xnp.tile([128, E], f32, tag="xn", name=f"wst{k}_{c}")
                    nc.sync.dma_start(wst, wv[:, k, c * E : (c + 1) * E])
                    nc.vector.tensor_copy(W[:, k, c * E : (c + 1) * E], wst)

            # bias broadcast across partitions (for [n-part, e] layouts)
            b3 = bvec[:].rearrange("(c m) -> c m", m=E)
            bb = constp.tile([128, 3, E], bf16)
            nc.gpsimd.dma_start(
                bb, bass.AP(tensor=b3.tensor, offset=b3.offset, ap=[[0, 128]] + list(b3.ap))
            )
            # bias per partition (for [e-part, n] layouts): bpart[p, c] = b[c*128+p]
            bpart = constp.tile([128, 3 * ED], f32)
            nc.gpsimd.dma_start(bpart, bvec[:].rearrange("(c p) -> p c", p=128))

            ident = constp.tile([128, 128], f32)
            make_identity(nc, ident)
            identB = constp.tile([128, 128], bf16)
            make_identity(nc, identB)

            def batch_body():
              for b in range(nb):
                qkt_dt = bf16 if "qkt_dma" in feat else f32r
                vn_dt = f32r if "f32r_pv" in feat else bf16
                dbufs = 2 if "dbuf" in feat else 1
                slice_xt = "slice_xt" in feat
                if not slice_xt:
                    xT = bigp.tile([128, ED, N], f32r, tag="xT")
                Qt = bigp.tile([128, ED, N], qkt_dt, tag="Qt")
                Kt = bigp.tile([128, ED, N], qkt_dt, tag="Kt")
                Vn = bigp.tile([128, NE, E], vn_dt, tag="Vn", bufs=dbufs)
                Vt = bigp.tile([128, ED, N], bf16, tag="Vt")
                Acol = bigp.tile([128, ED, E], bf16, tag="Acol", bufs=dbufs)
                AcolT = bigp.tile([128, ED, E], bf16, tag="AcolT", bufs=dbufs)
                scol_ps = [
                    ps_sc.tile([128, E], f32, tag="scps", name=f"scol{b}_{i}")
                    for i in range(ED)
                ]

                # ---- phase A: load x, build xT, projections, col-S accumulation
                for j in range(NE):
                    s_idx, jj = j // ED, j % ED
                    if slice_xt:
                        if jj == 0:
                            xT = xslp.tile(
                                [128, ED, ED * 128], f32r, tag="xsl", name=f"xsl{b}_{s_idx}"
                            )
                        xoff, roff = jj * 128, 0
                    else:
                        xoff, roff = j * 128, s_idx * 512
                    xn = xnp.tile([128, E], f32, tag="xn")
                    nc.sync.dma_start(xn, x[b, j * 128 : (j + 1) * 128, :])
                    for k in range(ED):
                        tps = ps_proj.tile([128, 128], f32, tag="ps")
                        nc.tensor.transpose(tps, xn[:, k * 128 : (k + 1) * 128], ident)
                        nc.vector.tensor_copy(xT[:, k, xoff : xoff + 128], tps)

                    # natural-layout q, k, v for this token chunk
                    qn = qknp.tile([128, E], bf16, tag="qn")
                    kn = qknp.tile([128, E], bf16, tag="kn")
                    for dst, ci in ((qn, 0), (kn, 1), (Vn[:, j, :], 2)):
                        pp = ps_proj.tile([128, E], f32, tag="ps")
                        for k in range(ED):
                            nc.tensor.matmul(
                                pp,
                                xT[:, k, xoff : xoff + 128],
                                W[:, k, ci * E : (ci + 1) * E],
                                start=(k == 0),
                                stop=(k == ED - 1),
                            )
                        nc.vector.tensor_add(dst, pp, bb[:, ci, :])

                    # col-attention S accumulation: S_col[d,e] += q_j.T @ k_j
                    for i in range(ED):
                        nc.tensor.matmul(
                            scol_ps[i],
                            qn[:, i * 128 : (i + 1) * 128],
                            kn,
                            start=(j == 0),
                            stop=(j == NE - 1),
                        )

                    # bf16 transposed layouts via the DMA XBAR (free wrt PE)
                    jsl = slice(j * 128, (j + 1) * 128)
                    if "vt_dma" in feat:
                        nc.scalar.dma_start_transpose(Vt[:, :, jsl], Vn[:, j, :])
                    if "qkt_dma" in feat:
                        nc.scalar.dma_start_transpose(Qt[:, :, jsl], qn)
                        nc.scalar.dma_start_transpose(Kt[:, :, jsl], kn)

                    # transposed-layout projections, one 512-token slice at a time
                    if "qkt_dma" in feat:
                        tproj = ()
                    elif "vt_dma" in feat:
                        tproj = ((Qt, 0), (Kt, 1))
                    else:
                        tproj = ((Qt, 0), (Kt, 1), (Vt, 2))
                    if j % ED == ED - 1 and tproj:
                        sl = slice(s_idx * 512, (s_idx + 1) * 512)
                        for dst, ci in tproj:
                            for i in range(ED):
                                pp = ps_proj.tile([128, E], f32, tag="ps")
                                for k in range(ED):
                                    nc.tensor.matmul(
                                        pp,
                                        W[:, k, ci * E + i * 128 : ci * E + (i + 1) * 128],
                                        xT[:, k, roff : roff + 512],
                                        start=(k == 0),
                                        stop=(k == ED - 1),
                                    )
                                if "act_drain" in feat:
                                    nc.scalar.activation(
                                        out=dst[:, i, sl],
                                        in_=pp,
                                        func=AF.Identity,
                                        bias=bpart[:, ci * ED + i : ci * ED + i + 1],
                                    )
                                else:
                                    nc.vector.tensor_scalar_add(
                                        dst[:, i, sl], pp, bpart[:, ci * ED + i : ci * ED + i + 1]
                                    )

                # ---- phase A2: col softmax + transpose of A
                # col logits are O(+-600): subtract the per-row max (as an ACT
                # bias of -max*SCALE) before exp, unlike the row path.
                cstat = statp.tile([128, 3 * ED], f32, tag="cstat")
                for i in range(ED):
                    nm = cstat[:, 2 * ED + i : 2 * ED + i + 1]
                    nc.vector.reduce_max(nm, scol_ps[i], axis=AX.X, negate=True)
                    nc.vector.tensor_scalar_mul(nm, nm, SCALE)
                    nc.scalar.activation(
                        out=Acol[:, i, :],
                        in_=scol_ps[i],
                        func=AF.Exp,
                        scale=SCALE,
                        bias=nm,
                        accum_out=cstat[:, i : i + 1],
                    )
                nc.vector.reciprocal(cstat[:, ED : 2 * ED], cstat[:, 0:ED])
                for i in range(ED):
                    nc.vector.tensor_scalar_mul(
                        Acol[:, i, :], Acol[:, i, :], cstat[:, ED + i : ED + i + 1]
                    )
                    nc.scalar.dma_start_transpose(
                        AcolT[:, :, i * 128 : (i + 1) * 128], Acol[:, i, :]
                    )

                # ---- phase B: row attention + merged output, per token chunk
                early_t = "early_t" in feat
                late_norm = "late_norm" in feat
                spread = "spread" in feat
                for j in range(NE):
                    teng = (nc.sync if j % 2 else nc.scalar) if spread else nc.scalar
                    yeng = (nc.scalar if j % 2 else nc.sync) if spread else nc.sync
                    pt = prowp.tile([128, N], bf16, tag="pt")
                    ptT = prowp.tile(
                        [128, NE, 128], f32r if "f32r_pv" in feat else bf16, tag="ptT"
                    )
                    rstat = statp.tile([128, 8], f32, tag="rstat")
                    for q in range(4):
                        sps = ps_s.tile([128, 512], f32, tag="s")
                        for k in range(ED):
                            nc.tensor.matmul(
                                sps,
                                Qt[:, k, j * 128 : (j + 1) * 128],
                                Kt[:, k, q * 512 : (q + 1) * 512],
                                start=(k == 0),
                                stop=(k == ED - 1),
                            )
                        nc.scalar.activation(
                            out=pt[:, q * 512 : (q + 1) * 512],
                            in_=sps,
                            func=AF.Exp,
                            scale=SCALE,
                            accum_out=rstat[:, q : q + 1],
                        )
                        if early_t:
                            # transpose the unnormalized quarter right away;
                            # 1/rowsum is applied to the PV output instead
                            teng.dma_start_transpose(
                                ptT[:, 4 * q : 4 * q + 4, :],
                                pt[:, q * 512 : (q + 1) * 512],
                            )
                        if "pe_pt" in feat:
                            for t in range(4):
                                m = 4 * q + t
                                psB = ps_proj.tile(
                                    [128, 128], bf16, tag="ps", name=f"psB{b}_{j}_{m}"
                                )
                                nc.tensor.transpose(
                                    psB, pt[:, m * 128 : (m + 1) * 128], identB
                                )
                                nc.vector.tensor_copy(ptT[:, m, :], psB)
                    nc.vector.reduce_sum(rstat[:, 4:5], rstat[:, 0:4], axis=AX.X)
                    nc.vector.reciprocal(rstat[:, 5:6], rstat[:, 4:5])
                    if late_norm:
                        if "pe_pt" not in feat:
                            teng.dma_start_transpose(ptT, pt)
                    elif not early_t:
                        nc.vector.tensor_scalar_mul(pt, pt, rstat[:, 5:6])
                        if "fake_t" in feat:
                            teng.dma_start(ptT.rearrange("p a b -> p (a b)"), pt)
                        elif "no_t" in feat:
                            nc.vector.tensor_copy(ptT[:, 0, :], pt[:, :128])
                        else:
                            teng.dma_start_transpose(ptT, pt)

                    po = ps_sc.tile([128, E], f32, tag="scps")
                    for m in range(NE):
                        nc.tensor.matmul(
                            po,
                            ptT[:, m, :],
                            Vn[:, m, :],
                            start=(m == 0),
                            stop=((early_t or late_norm) and m == NE - 1),
                        )
                    ot = outpp.tile([128, E], f32, tag="ot")
                    if early_t or late_norm:
                        oc = ps_sc.tile([128, E], f32, tag="scps")
                        for c in range(ED):
                            nc.tensor.matmul(
                                oc,
                                Vt[:, c, j * 128 : (j + 1) * 128],
                                AcolT[:, c, :],
                                start=(c == 0),
                                stop=(c == ED - 1),
                            )
                        if "dve_merge" in feat:
                            if "psum2" in feat:
                                nc.vector.scalar_tensor_tensor(
                                    ot,
                                    po,
                                    rstat[:, 5:6],
                                    oc,
                                    op0=mybir.AluOpType.mult,
                                    op1=mybir.AluOpType.add,
                                )
                            else:
                                octmp = outpp.tile([128, E], f32, tag="octmp")
                                nc.vector.tensor_copy(octmp, oc)
                                nc.vector.scalar_tensor_tensor(
                                    ot,
                                    po,
                                    rstat[:, 5:6],
                                    octmp,
                                    op0=mybir.AluOpType.mult,
                                    op1=mybir.AluOpType.add,
                                )
                        else:
                            nc.scalar.activation(
                                out=ot, in_=po, func=AF.Copy, scale=rstat[:, 5:6]
                            )
                            nc.vector.tensor_add(ot, ot, oc)
                    else:
                        for c in range(ED):
                            nc.tensor.matmul(
                                po,
                                Vt[:, c, j * 128 : (j + 1) * 128],
                                AcolT[:, c, :],
                                start=False,
                                stop=(c == ED - 1),
                            )
                        nc.vector.tensor_copy(ot, po)
                    yeng.dma_start(y[b, j * 128 : (j + 1) * 128, :], ot)

            if reps == 1:
                batch_body()
            else:
                with tc.For_i(0, reps, 1):
                    batch_body()

    nc.compile()
    _NC_CACHE[(nb, variant, reps)] = nc
    return nc


def build_nc_v30(nb=NB, variant=30, reps=1):
    """Restructured kernel: row-attention S computed TRANSPOSED (m on
    partitions) so exp(S.T) chunks feed the PV matmul as stationary operands
    directly — no P transposes at all.  Row softmax sums come from ones
    columns appended to V (PV computes [rowsum | out] in one accumulation).
    Qt/Kt come from PE transposes of the natural q/k projections (f32r in,
    bf16 out) instead of separate projection matmuls.

    variant 30: base (Qt/Kt bf16, full PT resident).
    variant 31: 30 but po0/po1 matmuls grouped per-stationary back to back.
    """
    if (nb, variant, reps) in _NC_CACHE:
        return _NC_CACHE[(nb, variant, reps)]

    import concourse.bass as bass
    import concourse.tile as tile
    from concourse import bacc, mybir
    from concourse.masks import make_identity

    f32 = mybir.dt.float32
    f32r = mybir.dt.float32r
    bf16 = mybir.dt.bfloat16
    AF = mybir.ActivationFunctionType
    AX = mybir.AxisListType

    nc = bacc.Bacc("TRN2")
    x = nc.dram_tensor("x", [nb, N, E], f32, kind="ExternalInput")
    w = nc.dram_tensor("w", [E, 3 * E], f32, kind="ExternalInput")
    bvec = nc.dram_tensor("b", [3 * E], f32, kind="ExternalInput")
    y = nc.dram_tensor("y", [nb, N, E], f32, kind="ExternalOutput")

    with tile.TileContext(nc) as tc:
        with (
            tc.tile_pool(name="const", bufs=1) as constp,
            tc.tile_pool(name="xn", bufs=3) as xnp,
            tc.tile_pool(name="wst", bufs=2) as wstp,
            tc.tile_pool(name="xtc", bufs=3) as xtcp,
            tc.tile_pool(name="qkn", bufs=3) as qknp,
            tc.tile_pool(name="big", bufs=1) as bigp,
            tc.tile_pool(name="stat", bufs=3) as statp,
            tc.tile_pool(name="outp", bufs=2) as outpp,
            tc.tile_pool(name="ps_proj", bufs=2, space="PSUM") as ps_proj,
            tc.tile_pool(name="ps_sc", bufs=4, space="PSUM") as ps_sc,
            tc.tile_pool(name="ps_s", bufs=2, space="PSUM") as ps_s,
        ):
            # ---------------- constants ----------------
            W = constp.tile([128, ED, 3 * E], f32r)
            wv = w[:].rearrange("(k p) m -> p k m", p=128)
            if variant >= 36:
                # c-major order so the q-projection weights are ready first;
                # DMAs on scalar/gpsimd queues (sync stays free for x), and
                # the f32r rounding copies round-robin DVE/Pool/ACT.
                wi = 0
                for c in range(3):
                    for k in range(ED):
                        wst = wstp.tile([128, E], f32, tag="wst", name=f"wst{k}_{c}")
                        (nc.scalar, nc.gpsimd)[wi % 2].dma_start(
                            wst, wv[:, k, c * E : (c + 1) * E]
                        )
                        dst = W[:, k, c * E : (c + 1) * E]
                        if wi % 3 == 0:
                            nc.vector.tensor_copy(dst, wst)
                        elif wi % 3 == 1:
                            nc.gpsimd.tensor_copy(dst, wst)
                        else:
                            nc.scalar.activation(out=dst, in_=wst, func=AF.Copy)
                        wi += 1
            else:
                wengs = (
                    (nc.sync, nc.scalar, nc.gpsimd)
                    if variant >= 34
                    else (nc.sync,)
                )
                for k in range(ED):
                    for c in range(3):
                        wst = xnp.tile([128, E], f32, tag="xn", name=f"wst{k}_{c}")
                        wengs[(k * 3 + c) % len(wengs)].dma_start(
                            wst, wv[:, k, c * E : (c + 1) * E]
                        )
                        nc.vector.tensor_copy(W[:, k, c * E : (c + 1) * E], wst)

            b3 = bvec[:].rearrange("(c m) -> c m", m=E)
            bb = constp.tile([128, 3, E], bf16)
            nc.gpsimd.dma_start(
                bb, bass.AP(tensor=b3.tensor, offset=b3.offset, ap=[[0, 128]] + list(b3.ap))
            )

            ident = constp.tile([128, 128], f32)
            make_identity(nc, ident)
            if variant < 36:
                identR = constp.tile([128, 128], f32r)
                make_identity(nc, identR)
            if variant >= 43:
                ones_b = constp.tile([128, 1], bf16)
                nc.gpsimd.memset(ones_b, 1.0)
                ones_f = constp.tile([128, 1], f32)
                nc.gpsimd.memset(ones_f, 1.0)

            def batch_body():
              for b in range(nb):
                v_w = 512 if variant >= 43 else 514
                Vx = bigp.tile([128, NE, v_w], bf16, tag="Vx")
                Qt = bigp.tile([128, ED, N], bf16, tag="Qt")
                Kt = bigp.tile([128, ED, N], bf16, tag="Kt")
                Vt = bigp.tile([128, ED, N], bf16, tag="Vt")
                PT = bigp.tile([128, NE, N], bf16, tag="PT")
                Acol = bigp.tile([128, ED, E], bf16, tag="Acol")
                AcolT = bigp.tile([128, ED, E], bf16, tag="AcolT")
                scol_ps = [
                    ps_sc.tile([128, E], f32, tag="scps", name=f"scol{b}_{i}")
                    for i in range(ED)
                ]
                if variant < 43:
                    # ones columns of Vx (positions 0 and 257) give the row
                    # sums of exp(S) as a byproduct of the PV accumulation
                    nc.gpsimd.memset(Vx[:, :, 0:1], 1.0)
                    nc.gpsimd.memset(Vx[:, :, 257:258], 1.0)

                # ---- phase A: load x, projections, col-S accumulation
                batched_t = variant >= 31
                pipe_xt = variant >= 33

                def load_xt(j):
                    """DMA chunk j of x and PE-transpose it into an xTc tile."""
                    xn = xnp.tile([128, E], f32, tag="xn")
                    nc.sync.dma_start(xn, x[b, j * 128 : (j + 1) * 128, :])
                    xTc = xtcp.tile([128, ED, 128], f32r, tag="xtc")
                    tg = ps_proj.tile([128, ED, 128], f32, tag="ps")
                    for k in range(ED):
                        nc.tensor.transpose(
                            tg[:, k, :], xn[:, k * 128 : (k + 1) * 128], ident
                        )
                    if variant >= 36:
                        nc.vector.tensor_copy(xTc, tg)
                    else:
                        nc.gpsimd.tensor_copy(xTc, tg)
                    return xTc

                if pipe_xt:
                    xTc_next = load_xt(0)
                for j in range(NE):
                    jsl = slice(j * 128, (j + 1) * 128)
                    if pipe_xt:
                        xTc = xTc_next
                        if j + 1 < NE:
                            xTc_next = load_xt(j + 1)
                    elif batched_t:
                        xTc = load_xt(j)
                    else:
                        xn = xnp.tile([128, E], f32, tag="xn")
                        nc.sync.dma_start(xn, x[b, jsl, :])
                        xTc = xtcp.tile([128, ED, 128], f32r, tag="xtc")
                        for k in range(ED):
                            tps = ps_proj.tile([128, 128], f32, tag="ps")
                            nc.tensor.transpose(
                                tps, xn[:, k * 128 : (k + 1) * 128], ident
                            )
                            nc.gpsimd.tensor_copy(xTc[:, k, :], tps)

                    qn = qknp.tile([128, E], f32r, tag="qn")
                    kn = qknp.tile([128, E], f32r, tag="kn")
                    if variant >= 32:
                        qnb = qknp.tile([128, E], bf16, tag="qnb")
                        knb = qknp.tile([128, E], bf16, tag="knb")
                    for dst, ci in ((qn, 0), (kn, 1)):
                        pp = ps_proj.tile([128, E], f32, tag="ps")
                        for k in range(ED):
                            nc.tensor.matmul(
                                pp,
                                xTc[:, k, :],
                                W[:, k, ci * E : (ci + 1) * E],
                                start=(k == 0),
                                stop=(k == ED - 1),
                            )
                        nc.vector.tensor_add(dst, pp, bb[:, ci, :])
                        if variant >= 33:
                            # bf16 copy (off the PSUM critical path: reads SBUF)
                            nc.gpsimd.tensor_copy((qnb, knb)[ci], dst)
                        elif variant >= 32:
                            # bf16 copy for the DMA-XBAR transpose into Qt/Kt
                            nc.gpsimd.tensor_add(
                                (qnb, knb)[ci], pp, bb[:, ci, :]
                            )
                    pp = ps_proj.tile([128, E], f32, tag="ps")
                    for k in range(ED):
                        nc.tensor.matmul(
                            pp,
                            xTc[:, k, :],
                            W[:, k, 2 * E : 3 * E],
                            start=(k == 0),
                            stop=(k == ED - 1),
                        )
                    veng = nc.vector if variant >= 36 else nc.gpsimd
                    if variant >= 43:
                        veng.tensor_add(Vx[:, j, :], pp, bb[:, 2, :])
                    else:
                        veng.tensor_add(Vx[:, j, 1:257], pp[:, 0:256], bb[:, 2, 0:256])
                        veng.tensor_add(Vx[:, j, 258:514], pp[:, 256:512], bb[:, 2, 256:512])

                    # col-attention S accumulation (f32r operands for precision)
                    for i in range(ED):
                        nc.tensor.matmul(
                            scol_ps[i],
                            qn[:, i * 128 : (i + 1) * 128],
                            kn,
                            start=(j == 0),
                            stop=(j == NE - 1),
                        )

                    # Qt/Kt via PE transpose (f32r 1.5 cyc/row), drained bf16
                    if variant >= 32:
                        qeng = nc.sync if variant >= 34 else nc.scalar
                        keng = nc.scalar
                        if variant != 41 or j == 0:
                            for h in range(2):
                                qeng.dma_start_transpose(
                                    Qt[:, 2 * h : 2 * h + 2, jsl],
                                    qnb[:, h * 256 : (h + 1) * 256],
                                )
                                keng.dma_start_transpose(
                                    Kt[:, 2 * h : 2 * h + 2, jsl],
                                    knb[:, h * 256 : (h + 1) * 256],
                                )
                    elif batched_t:
                        for src, dstT, deng in ((qn, Qt, 0), (kn, Kt, 1)):
                            tg = ps_proj.tile([128, ED, 128], f32r, tag="ps")
                            for k in range(ED):
                                nc.tensor.transpose(
                                    tg[:, k, :], src[:, k * 128 : (k + 1) * 128], identR
                                )
                            if deng == 0:
                                nc.scalar.activation(
                                    out=dstT[:, :, jsl], in_=tg, func=AF.Copy
                                )
                            else:
                                nc.vector.tensor_copy(dstT[:, :, jsl], tg)
                    else:
                        for src, dstT in ((qn, Qt), (kn, Kt)):
                            for k in range(ED):
                                tpr = ps_proj.tile([128, 128], f32r, tag="ps")
                                nc.tensor.transpose(
                                    tpr, src[:, k * 128 : (k + 1) * 128], identR
                                )
                                nc.scalar.activation(
                                    out=dstT[:, k, jsl], in_=tpr, func=AF.Copy
                                )

                    # Vt via DMA XBAR transpose
                    if variant >= 43:
                        nc.scalar.dma_start_transpose(Vt[:, :, jsl], Vx[:, j, :])
                    else:
                        for h in range(2):
                            nc.scalar.dma_start_transpose(
                                Vt[:, 2 * h : 2 * h + 2, jsl],
                                Vx[:, j, h * 257 + 1 : h * 257 + 257],
                            )

                # ---- phase A2: col softmax + transpose of A
                import contextlib
                a2ctx = tc.high_priority() if variant >= 37 else contextlib.nullcontext()
                with a2ctx:
                    cstat = statp.tile([128, 3 * ED], f32, tag="cstat")
                    for i in range(ED):
                        nm = cstat[:, 2 * ED + i : 2 * ED + i + 1]
                        nc.vector.reduce_max(nm, scol_ps[i], axis=AX.X, negate=True)
                        nc.vector.tensor_scalar_mul(nm, nm, SCALE)
                        nc.scalar.activation(
                            out=Acol[:, i, :],
                            in_=scol_ps[i],
                            func=AF.Exp,
                            scale=SCALE,
                            bias=nm,
                            accum_out=cstat[:, i : i + 1],
                        )
                    nc.vector.reciprocal(cstat[:, ED : 2 * ED], cstat[:, 0:ED])
                    for i in range(ED):
                        nc.vector.tensor_scalar_mul(
                            Acol[:, i, :], Acol[:, i, :], cstat[:, ED + i : ED + i + 1]
                        )
                        nc.scalar.dma_start_transpose(
                            AcolT[:, :, i * 128 : (i + 1) * 128], Acol[:, i, :]
                        )

                # ---- phase B1: S.T per m-chunk + exp straight into PT
                for mc in range(NE):
                    for q in range(ED):
                        sps = ps_s.tile([128, 512], f32, tag="s")
                        for k in range(ED):
                            nc.tensor.matmul(
                                sps,
                                Kt[:, k, mc * 128 : (mc + 1) * 128],
                                Qt[:, k, q * 512 : (q + 1) * 512],
                                start=(k == 0),
                                stop=(k == ED - 1),
                            )
                        nc.scalar.activation(
                            out=PT[:, mc, q * 512 : (q + 1) * 512],
                            in_=sps,
                            func=AF.Exp,
                            scale=SCALE,
                        )

                # ---- rowsum pass (v43+): rowsum_n = sum_m exp(S.T)[m, n]
                # via ones-stationary matmuls, then tiny on-chip transposes
                # to land 1/rowsum per token partition.
                if variant >= 43:
                    rsT = [
                        ps_sc.tile([128, 512], f32, tag="scps", name=f"rs{b}_{t}")
                        for t in range(2)
                    ]
                    for q4 in range(ED):
                        row = 32 * (q4 % 2)
                        nc_rs = rsT[q4 // 2]
                        for mc in range(NE):
                            nc.tensor.matmul(
                                nc_rs[row : row + 1, :],
                                ones_b,
                                PT[:, mc, q4 * 512 : (q4 + 1) * 512],
                                start=(mc == 0),
                                stop=(mc == NE - 1),
                            )
                    rss = [statp.tile([128, 512], f32, tag="rss", name=f"rss{b}_{t}", bufs=1) for t in range(2)]
                    for t in range(2):
                        for row in (0, 32):
                            nc.vector.tensor_copy(
                                rss[t][row : row + 1, :], rsT[t][row : row + 1, :]
                            )
                    rt_ps = ps_sc.tile([128, 16], f32, tag="scps", name=f"rt{b}")
                    for g in range(NE):
                        q4, jl = g // 4, g % 4
                        row = 32 * (q4 % 2)
                        nc.tensor.matmul(
                            rt_ps[:, g : g + 1],
                            rss[q4 // 2][row : row + 1, jl * 128 : (jl + 1) * 128],
                            ones_f[row : row + 1, :],
                            start=True,
                            stop=True,
                        )
                    rstatR = statp.tile([128, NE], f32, tag="rstatR", bufs=1)
                    nc.vector.reciprocal(rstatR, rt_ps)

                # ---- phase B2: PV + col output + merge
                for j in range(NE):
                    jsl = slice(j * 128, (j + 1) * 128)
                    if variant >= 43:
                        po0 = ps_sc.tile([128, E], f32, tag="scps")
                        for mc in range(NE):
                            nc.tensor.matmul(
                                po0,
                                PT[:, mc, jsl],
                                Vx[:, mc, :],
                                start=(mc == 0),
                                stop=(mc == NE - 1),
                            )
                    else:
                        po0 = ps_sc.tile([128, 257], f32, tag="scps")
                        po1 = ps_sc.tile([128, 257], f32, tag="scps")
                        for mc in range(NE):
                            nc.tensor.matmul(
                                po0,
                                PT[:, mc, jsl],
                                Vx[:, mc, 0:257],
                                start=(mc == 0),
                                stop=(mc == NE - 1),
                            )
                            if variant != 40:  # 40: timing probe, drops po1
                                nc.tensor.matmul(
                                    po1,
                                    PT[:, mc, jsl],
                                    Vx[:, mc, 257:514],
                                    start=(mc == 0),
                                    stop=(mc == NE - 1),
                                )
                        if variant == 40:
                            nc.tensor.matmul(
                                po1, PT[:, 0, jsl], Vx[:, 0, 257:514],
                                start=True, stop=True,
                            )
                    oc = ps_s.tile([128, E], f32, tag="s")
                    for c in range(ED):
                        nc.tensor.matmul(
                            oc,
                            Vt[:, c, jsl],
                            AcolT[:, c, :],
                            start=(c == 0),
                            stop=(c == ED - 1),
                        )
                    ot = outpp.tile([128, E], f32, tag="ot")
                    octmp = outpp.tile([128, E], f32, tag="octmp")
                    if 31 <= variant < 36:
                        nc.gpsimd.tensor_copy(octmp, oc)
                    else:
                        nc.scalar.activation(out=octmp, in_=oc, func=AF.Copy)
                    if variant >= 43:
                        nc.vector.scalar_tensor_tensor(
                            ot,
                            po0,
                            rstatR[:, j : j + 1],
                            octmp,
                            op0=mybir.AluOpType.mult,
                            op1=mybir.AluOpType.add,
                        )
                    else:
                        rstat = statp.tile([128, 1], f32, tag="rstat")
                        nc.vector.reciprocal(rstat, po0[:, 0:1])
                        nc.vector.scalar_tensor_tensor(
                            ot[:, 0:256],
                            po0[:, 1:257],
                            rstat,
                            octmp[:, 0:256],
                            op0=mybir.AluOpType.mult,
                            op1=mybir.AluOpType.add,
                        )
                        nc.vector.scalar_tensor_tensor(
                            ot[:, 256:512],
                            po1[:, 1:257],
                            rstat,
                            octmp[:, 256:512],
                            op0=mybir.AluOpType.mult,
                            op1=mybir.AluOpType.add,
                        )
                    nc.sync.dma_start(y[b, jsl, :], ot)

            if reps == 1:
                batch_body()
            else:
                with tc.For_i(0, reps, 1):
                    batch_body()

    nc.compile()
    _NC_CACHE[(nb, variant, reps)] = nc
    return nc


def make_in_maps(x, w_qkv, b_qkv):
    xs = np.ascontiguousarray(np.asarray(x, dtype=np.float32)).reshape(B, N, E)
    w = np.ascontiguousarray(np.asarray(w_qkv, dtype=np.float32))
    bq = np.ascontiguousarray(np.asarray(b_qkv, dtype=np.float32))
    return [
        {"x": np.ascontiguousarray(xs[c * NB : (c + 1) * NB]), "w": w, "b": bq}
        for c in range(NCORES)
    ]


BEST_VARIANT = 37


def kernel(x, w_qkv, b_qkv):
    from concourse.bass_utils import run_bass_kernel_spmd

    nc = build_nc(NB, BEST_VARIANT)
    in_maps = make_in_maps(x, w_qkv, b_qkv)
    res = run_bass_kernel_spmd(nc, in_maps, core_ids=list(range(NCORES)))
    out = np.empty((B, N, E), dtype=np.float32)
    for c in range(NCORES):
        out[c * NB : (c + 1) * NB] = res.results[c]["y"]
    return out



# revision 43
# speedup vs baseline: 40.4290x; 1.0734x over previous
"""AxialSelfAttention Trainium2 Bass kernel.

Reference computation (per batch b):
    xs  = x[b] reshaped [N=2048, E=512]
    qkv = xs @ W + bias                      # [N, 3E]
    q, k, v = split(qkv)
    row:  P = softmax(q @ k.T / sqrt(E));  out_row = P @ v
    col:  A = softmax(q.T @ k / sqrt(E));  out_col = v @ A.T
    out = out_row + out_col                  # [N, E]

Sharding: data-parallel over batch B=32 across 8 cores (4 batches/core).

Best variant (45) per-core layout strategy:
  - Phase A, per 128-token chunk: x loaded naturally, PE-transposed one
    chunk ahead (pipelined) into per-chunk xT slices (f32r); natural
    q/k/v projections from xT (f32r matmuls, full precision for the
    col path); col-attention S accumulated in 4 persistent PSUM banks
    across all 16 chunks.
  - Qt/Kt (row-S operands, [e-part, n] bf16) come from DMA-XBAR
    transposes of bf16 copies of q/k -- zero PE cost, latency hidden
    because row-S only starts once 4 chunks are available.  Vt
    likewise.  The ones columns inside the padded V tile (positions 0
    and 257) make the PV matmul emit the softmax row sums directly.
  - Phase B1 computes row-attention S TRANSPOSED (S.T[m, n], m on
    partitions): exp(S.T) chunks land in SBUF as PT and feed the PV
    matmul as stationary operands directly -- no P transposes at all.
  - Phase B2, per token chunk: PV accumulates [rowsum | out_row] via
    two 257-wide PSUM chains; the col output accumulates from Vt @
    AcolT; one DVE scalar_tensor_tensor merges out_row/rowsum +
    out_col.
  - The col softmax (A2) runs at high scheduler priority so its PSUM
    banks free early for B2; softmax max-subtraction is skipped on the
    row path (logits are O(+-10)) but kept on the col path (O(+-600)).
Measured on HW (8 cores, marginal time per kernel invocation via
hardware-loop reps): ~1.19 ms vs 1.44-1.56 ms for the previous best
(variant 13) in the same session.
"""

import sys

for _p in ("/opt/trn_rl_repo", "/root/.axon_site/_ro/trn_rl_repo"):
    if _p not in sys.path:
        sys.path.append(_p)

import numpy as np

B, N, E = 32, 2048, 512
NCORES = 8
NB = B // NCORES  # batches per core
NE = N // 128  # 16 token chunks
ED = E // 128  # 4 feature chunks
SCALE = 1.0 / float(np.sqrt(E))

_NC_CACHE = {}


def build_nc(nb=NB, variant=13, reps=1):
    """Build (once) the single-core Bass module processing nb batches.

    variant 1: all six projection layouts via fp32r matmuls from xT.
    variant 2: like 1, but Vt comes from a bf16 DMA(XBAR)-transpose of V
               instead of its own matmul projection (-64 matmuls/batch).
    variant 3: like 2, and Qt/Kt also come from DMA-transposes of the bf16
               natural q/k (row-attention S then runs in bf16;
               -128 more matmuls/batch).
    variant 4: like 3, plus V/Acol/AcolT double-buffered across batches so
               batch b+1's projection phase (PE) can stream while batch b's
               row attention still reads V.
    variant 5: variant 2 + early transposes: exp quarters are transposed
               unnormalized as soon as they exist, and the 1/rowsum lands on
               the PV output (per-partition ACT scale) instead of on P~;
               PV and the col output use separate PSUM tiles.
    variant 6: variant 5 + the double-buffering of variant 4.
    variant 7: variant 6 + 3-deep prow pool.
    """
    FEAT = {
        1: set(),
        2: {"vt_dma"},
        3: {"vt_dma", "qkt_dma"},
        4: {"vt_dma", "qkt_dma", "dbuf"},
        5: {"vt_dma", "early_t"},
        6: {"vt_dma", "early_t", "dbuf"},
        7: {"vt_dma", "early_t", "dbuf", "prow3"},
        8: {"vt_dma", "spread"},
        9: {"vt_dma", "early_t", "spread"},
        10: {"vt_dma", "late_norm", "act_drain"},
        11: {"vt_dma", "act_drain"},
        13: {"vt_dma", "late_norm", "pe_pt", "dve_merge"},
        14: {"vt_dma", "late_norm", "dve_merge"},
        # timing-only diagnostics (wrong results): fake / absent pt transposes
        15: {"vt_dma", "fake_t"},
        16: {"vt_dma", "no_t"},
        17: {"late_norm", "pe_pt", "dve_merge"},
        18: {"vt_dma", "late_norm", "pe_pt", "dve_merge", "prow3"},
        20: {"vt_dma", "late_norm", "pe_pt", "dve_merge", "psum2"},
        22: {"late_norm", "pe_pt", "dve_merge", "f32r_pv", "slice_xt"},
        23: {"late_norm", "pe_pt", "dve_merge", "f32r_pv", "slice_xt", "prow3"},
    }
    if variant >= 30:
        return build_nc_v30(nb, variant, reps)
    feat = FEAT[variant]
    if (nb, variant, reps) in _NC_CACHE:
        return _NC_CACHE[(nb, variant, reps)]

    import concourse.bass as bass
    import concourse.tile as tile
    from concourse import bacc, mybir
    from concourse.masks import make_identity

    f32 = mybir.dt.float32
    f32r = mybir.dt.float32r
    bf16 = mybir.dt.bfloat16
    AF = mybir.ActivationFunctionType
    AX = mybir.AxisListType

    nc = bacc.Bacc("TRN2")
    x = nc.dram_tensor("x", [nb, N, E], f32, kind="ExternalInput")
    w = nc.dram_tensor("w", [E, 3 * E], f32, kind="ExternalInput")
    bvec = nc.dram_tensor("b", [3 * E], f32, kind="ExternalInput")
    y = nc.dram_tensor("y", [nb, N, E], f32, kind="ExternalOutput")

    with tile.TileContext(nc) as tc:
        with (
            tc.tile_pool(name="const", bufs=1) as constp,
            tc.tile_pool(name="xn", bufs=2 if ("prow3" in feat or "slice_xt" in feat) else 3) as xnp,
            tc.tile_pool(name="big", bufs=1) as bigp,
            tc.tile_pool(name="xsl", bufs=2) as xslp,
            tc.tile_pool(name="qkn", bufs=3) as qknp,
            tc.tile_pool(name="prow", bufs=3 if "prow3" in feat else 2) as prowp,
            tc.tile_pool(name="stat", bufs=3) as statp,
            tc.tile_pool(name="outp", bufs=2) as outpp,
            tc.tile_pool(name="ps_proj", bufs=2, space="PSUM") as ps_proj,
            tc.tile_pool(name="ps_sc", bufs=4, space="PSUM") as ps_sc,
            tc.tile_pool(name="ps_s", bufs=2, space="PSUM") as ps_s,
        ):
            # ---------------- constants ----------------
            # W lands as float32r (rounded by the DVE copy) so fp32r matmuls
            # accept it; staged through the small xn pool to save SBUF.
            W = constp.tile([128, ED, 3 * E], f32r)
            wv = w[:].rearrange("(k p) m -> p k m", p=128)
            for k in range(ED):
                for c in range(3):
                    wst = xnp.tile([128, E], f32, tag="xn", name=f"wst{k}_{c}")
                    nc.sync.dma_start(wst, wv[:, k, c * E : (c + 1) * E])
                    nc.vector.tensor_copy(W[:, k, c * E : (c + 1) * E], wst)

            # bias broadcast across partitions (for [n-part, e] layouts)
            b3 = bvec[:].rearrange("(c m) -> c m", m=E)
            bb = constp.tile([128, 3, E], bf16)
            nc.gpsimd.dma_start(
                bb, bass.AP(tensor=b3.tensor, offset=b3.offset, ap=[[0, 128]] + list(b3.ap))
            )
            # bias per partition (for [e-part, n] layouts): bpart[p, c] = b[c*128+p]
            bpart = constp.tile([128, 3 * ED], f32)
            nc.gpsimd.dma_start(bpart, bvec[:].rearrange("(c p) -> p c", p=128))

            ident = constp.tile([128, 128], f32)
            make_identity(nc, ident)
            identB = constp.tile([128, 128], bf16)
            make_identity(nc, identB)

            def batch_body():
              for b in range(nb):
                qkt_dt = bf16 if "qkt_dma" in feat else f32r
                vn_dt = f32r if "f32r_pv" in feat else bf16
                dbufs = 2 if "dbuf" in feat else 1
                slice_xt = "slice_xt" in feat
                if not slice_xt:
                    xT = bigp.tile([128, ED, N], f32r, tag="xT")
                Qt = bigp.tile([128, ED, N], qkt_dt, tag="Qt")
                Kt = bigp.tile([128, ED, N], qkt_dt, tag="Kt")
                Vn = bigp.tile([128, NE, E], vn_dt, tag="Vn", bufs=dbufs)
                Vt = bigp.tile([128, ED, N], bf16, tag="Vt")
                Acol = bigp.tile([128, ED, E], bf16, tag="Acol", bufs=dbufs)
                AcolT = bigp.tile([128, ED, E], bf16, tag="AcolT", bufs=dbufs)
                scol_ps = [
                    ps_sc.tile([128, E], f32, tag="scps", name=f"scol{b}_{i}")
                    for i in range(ED)
                ]

                # ---- phase A: load x, build xT, projections, col-S accumulation
                for j in range(NE):
                    s_idx, jj = j // ED, j % ED
                    if slice_xt:
                        if jj == 0:
                            xT = xslp.tile(
                                [128, ED, ED * 128], f32r, tag="xsl", name=f"xsl{b}_{s_idx}"
                            )
                        xoff, roff = jj * 128, 0
                    else:
                        xoff, roff = j * 128, s_idx * 512
                    xn = xnp.tile([128, E], f32, tag="xn")
                    nc.sync.dma_start(xn, x[b, j * 128 : (j + 1) * 128, :])
                    for k in range(ED):
                        tps = ps_proj.tile([128, 128], f32, tag="ps")
                        nc.tensor.transpose(tps, xn[:, k * 128 : (k + 1) * 128], ident)
                        nc.vector.tensor_copy(xT[:, k, xoff : xoff + 128], tps)

                    # natural-layout q, k, v for this token chunk
                    qn = qknp.tile([128, E], bf16, tag="qn")
                    kn = qknp.tile([128, E], bf16, tag="kn")
                    for dst, ci in ((qn, 0), (kn, 1), (Vn[:, j, :], 2)):
                        pp = ps_proj.tile([128, E], f32, tag="ps")
                        for k in range(ED):
                            nc.tensor.matmul(
                                pp,
                                xT[:, k, xoff : xoff + 128],
                                W[:, k, ci * E : (ci + 1) * E],
                                start=(k == 0),
                                stop=(k == ED - 1),
                            )
                        nc.vector.tensor_add(dst, pp, bb[:, ci, :])

                    # col-attention S accumulation: S_col[d,e] += q_j.T @ k_j
                    for i in range(ED):
                        nc.tensor.matmul(
                            scol_ps[i],
                            qn[:, i * 128 : (i + 1) * 128],
                            kn,
                            start=(j == 0),
                            stop=(j == NE - 1),
                        )

                    # bf16 transposed layouts via the DMA XBAR (free wrt PE)
                    jsl = slice(j * 128, (j + 1) * 128)
                    if "vt_dma" in feat:
                        nc.scalar.dma_start_transpose(Vt[:, :, jsl], Vn[:, j, :])
                    if "qkt_dma" in feat:
                        nc.scalar.dma_start_transpose(Qt[:, :, jsl], qn)
                        nc.scalar.dma_start_transpose(Kt[:, :, jsl], kn)

                    # transposed-layout projections, one 512-token slice at a time
                    if "qkt_dma" in feat:
                        tproj = ()
                    elif "vt_dma" in feat:
                        tproj = ((Qt, 0), (Kt, 1))
                    else:
                        tproj = ((Qt, 0), (Kt, 1), (Vt, 2))
                    if j % ED == ED - 1 and tproj:
                        sl = slice(s_idx * 512, (s_idx + 1) * 512)
                        for dst, ci in tproj:
                            for i in range(ED):
                                pp = ps_proj.tile([128, E], f32, tag="ps")
                                for k in range(ED):
                                    nc.tensor.matmul(
                                        pp,
                                        W[:, k, ci * E + i * 128 : ci * E + (i + 1) * 128],
                                        xT[:, k, roff : roff + 512],
                                        start=(k == 0),
                                        stop=(k == ED - 1),
                                    )
                                if "act_drain" in feat:
                                    nc.scalar.activation(
                                        out=dst[:, i, sl],
                                        in_=pp,
                                        func=AF.Identity,
                                        bias=bpart[:, ci * ED + i : ci * ED + i + 1],
                                    )
                                else:
                                    nc.vector.tensor_scalar_add(
                                        dst[:, i, sl], pp, bpart[:, ci * ED + i : ci * ED + i + 1]
                                    )

                # ---- phase A2: col softmax + transpose of A
                # col logits are O(+-600): subtract the per-row max (as an ACT
                # bias of -max*SCALE) before exp, unlike the row path.
                cstat = statp.tile([128, 3 * ED], f32, tag="cstat")
                for i in range(ED):
                    nm = cstat[:, 2 * ED + i : 2 * ED + i + 1]
                    nc.vector.reduce_max(nm, scol_ps[i], axis=AX.X, negate=True)
                    nc.vector.tensor_scalar_mul(nm, nm, SCALE)
                    nc.scalar.activation(
                        out=Acol[:, i, :],
                        in_=scol_ps[i],
                        func=AF.Exp,
                        scale=SCALE,
                        bias=nm,
                        accum_out=cstat[:, i : i + 1],
                    )
                nc.vector.reciprocal(cstat[:, ED : 2 * ED], cstat[:, 0:ED])
                for i in range(ED):
                    nc.vector.tensor_scalar_mul(
                        Acol[:, i, :], Acol[:, i, :], cstat[:, ED + i : ED + i + 1]
                    )
                    nc.scalar.dma_start_transpose(
                        AcolT[:, :, i * 128 : (i + 1) * 128], Acol[:, i, :]
                    )

                # ---- phase B: row attention + merged output, per token chunk
                early_t = "early_t" in feat
                late_norm = "late_norm" in feat
                spread = "spread" in feat
                for j in range(NE):
                    teng = (nc.sync if j % 2 else nc.scalar) if spread else nc.scalar
                    yeng = (nc.scalar if j % 2 else nc.sync) if spread else nc.sync
                    pt = prowp.tile([128, N], bf16, tag="pt")
                    ptT = prowp.tile(
                        [128, NE, 128], f32r if "f32r_pv" in feat else bf16, tag="ptT"
                    )
                    rstat = statp.tile([128, 8], f32, tag="rstat")
                    for q in range(4):
                        sps = ps_s.tile([128, 512], f32, tag="s")
                        for k in range(ED):
                            nc.tensor.matmul(
                                sps,
                                Qt[:, k, j * 128 : (j + 1) * 128],
                                Kt[:, k, q * 512 : (q + 1) * 512],
                                start=(k == 0),
                                stop=(k == ED - 1),
                            )
                        nc.scalar.activation(
                            out=pt[:, q * 512 : (q + 1) * 512],
                            in_=sps,
                            func=AF.Exp,
                            scale=SCALE,
                            accum_out=rstat[:, q : q + 1],
                        )
                        if early_t:
                            # transpose the unnormalized quarter right away;
                            # 1/rowsum is applied to the PV output instead
                            teng.dma_start_transpose(
                                ptT[:, 4 * q : 4 * q + 4, :],
                                pt[:, q * 512 : (q + 1) * 512],
                            )
                        if "pe_pt" in feat:
                            for t in range(4):
                                m = 4 * q + t
                                psB = ps_proj.tile(
                                    [128, 128], bf16, tag="ps", name=f"psB{b}_{j}_{m}"
                                )
                                nc.tensor.transpose(
                                    psB, pt[:, m * 128 : (m + 1) * 128], identB
                                )
                                nc.vector.tensor_copy(ptT[:, m, :], psB)
                    nc.vector.reduce_sum(rstat[:, 4:5], rstat[:, 0:4], axis=AX.X)
                    nc.vector.reciprocal(rstat[:, 5:6], rstat[:, 4:5])
                    if late_norm:
                        if "pe_pt" not in feat:
                            teng.dma_start_transpose(ptT, pt)
                    elif not early_t:
                        nc.vector.tensor_scalar_mul(pt, pt, rstat[:, 5:6])
                        if "fake_t" in feat:
                            teng.dma_start(ptT.rearrange("p a b -> p (a b)"), pt)
                        elif "no_t" in feat:
                            nc.vector.tensor_copy(ptT[:, 0, :], pt[:, :128])
                        else:
                            teng.dma_start_transpose(ptT, pt)

                    po = ps_sc.tile([128, E], f32, tag="scps")
                    for m in range(NE):
                        nc.tensor.matmul(
                            po,
                            ptT[:, m, :],
                            Vn[:, m, :],
                            start=(m == 0),
                            stop=((early_t or late_norm) and m == NE - 1),
                        )
                    ot = outpp.tile([128, E], f32, tag="ot")
                    if early_t or late_norm:
                        oc = ps_sc.tile([128, E], f32, tag="scps")
                        for c in range(ED):
                            nc.tensor.matmul(
                                oc,
                                Vt[:, c, j * 128 : (j + 1) * 128],
                                AcolT[:, c, :],
                                start=(c == 0),
                                stop=(c == ED - 1),
                            )
                        if "dve_merge" in feat:
                            if "psum2" in feat:
                                nc.vector.scalar_tensor_tensor(
                                    ot,
                                    po,
                                    rstat[:, 5:6],
                                    oc,
                                    op0=mybir.AluOpType.mult,
                                    op1=mybir.AluOpType.add,
                                )
                            else:
                                octmp = outpp.tile([128, E], f32, tag="octmp")
                                nc.vector.tensor_copy(octmp, oc)
                                nc.vector.scalar_tensor_tensor(
                                    ot,
                                    po,
                                    rstat[:, 5:6],
                                    octmp,
                                    op0=mybir.AluOpType.mult,
                                    op1=mybir.AluOpType.add,
                                )
                        else:
                            nc.scalar.activation(
                                out=ot, in_=po, func=AF.Copy, scale=rstat[:, 5:6]
                            )
                            nc.vector.tensor_add(ot, ot, oc)
                    else:
                        for c in range(ED):
                            nc.tensor.matmul(
                                po,
                                Vt[:, c, j * 128 : (j + 1) * 128],
                                AcolT[:, c, :],
                                start=False,
                                stop=(c == ED - 1),
                            )
                        nc.vector.tensor_copy(ot, po)
                    yeng.dma_start(y[b, j * 128 : (j + 1) * 128, :], ot)

            if reps == 1:
                batch_body()
            else:
                with tc.For_i(0, reps, 1):
                    batch_body()

    nc.compile()
    _NC_CACHE[(nb, variant, reps)] = nc
    return nc


def build_nc_v30(nb=NB, variant=30, reps=1):
    """Restructured kernel: row-attention S computed TRANSPOSED (m on
    partitions) so exp(S.T) chunks feed the PV matmul as stationary operands
    directly — no P transposes at all.  Row softmax sums come from ones
    columns appended to V (PV computes [rowsum | out] in one accumulation).
    Qt/Kt come from PE transposes of the natural q/k projections (f32r in,
    bf16 out) instead of separate projection matmuls.

    variant 30: base (Qt/Kt bf16, full PT resident).
    variant 31: 30 but po0/po1 matmuls grouped per-stationary back to back.
    """
    if (nb, variant, reps) in _NC_CACHE:
        return _NC_CACHE[(nb, variant, reps)]

    import concourse.bass as bass
    import concourse.tile as tile
    from concourse import bacc, mybir
    from concourse.masks import make_identity

    f32 = mybir.dt.float32
    f32r = mybir.dt.float32r
    bf16 = mybir.dt.bfloat16
    AF = mybir.ActivationFunctionType
    AX = mybir.AxisListType

    nc = bacc.Bacc("TRN2")
    x = nc.dram_tensor("x", [nb, N, E], f32, kind="ExternalInput")
    w = nc.dram_tensor("w", [E, 3 * E], f32, kind="ExternalInput")
    bvec = nc.dram_tensor("b", [3 * E], f32, kind="ExternalInput")
    y = nc.dram_tensor("y", [nb, N, E], f32, kind="ExternalOutput")

    with tile.TileContext(nc) as tc:
        with (
            tc.tile_pool(name="const", bufs=1) as constp,
            tc.tile_pool(name="xn", bufs=3) as xnp,
            tc.tile_pool(name="wst", bufs=2) as wstp,
            tc.tile_pool(name="xtc", bufs=3) as xtcp,
            tc.tile_pool(name="qkn", bufs=3) as qknp,
            tc.tile_pool(name="big", bufs=1) as bigp,
            tc.tile_pool(name="stat", bufs=3) as statp,
            tc.tile_pool(name="outp", bufs=2) as outpp,
            tc.tile_pool(name="ps_proj", bufs=2, space="PSUM") as ps_proj,
            tc.tile_pool(name="ps_sc", bufs=4, space="PSUM") as ps_sc,
            tc.tile_pool(name="ps_s", bufs=2, space="PSUM") as ps_s,
        ):
            # ---------------- constants ----------------
            W = constp.tile([128, ED, 3 * E], f32r)
            wv = w[:].rearrange("(k p) m -> p k m", p=128)
            if variant >= 36:
                # c-major order so the q-projection weights are ready first;
                # DMAs on scalar/gpsimd queues (sync stays free for x), and
                # the f32r rounding copies round-robin DVE/Pool/ACT.
                wi = 0
                for c in range(3):
                    for k in range(ED):
                        wst = wstp.tile([128, E], f32, tag="wst", name=f"wst{k}_{c}")
                        (nc.scalar, nc.gpsimd)[wi % 2].dma_start(
                            wst, wv[:, k, c * E : (c + 1) * E]
                        )
                        dst = W[:, k, c * E : (c + 1) * E]
                        if wi % 3 == 0:
                            nc.vector.tensor_copy(dst, wst)
                        elif wi % 3 == 1:
                            nc.gpsimd.tensor_copy(dst, wst)
                        else:
                            nc.scalar.activation(out=dst, in_=wst, func=AF.Copy)
                        wi += 1
            else:
                wengs = (
                    (nc.sync, nc.scalar, nc.gpsimd)
                    if variant >= 34
                    else (nc.sync,)
                )
                for k in range(ED):
                    for c in range(3):
                        wst = xnp.tile([128, E], f32, tag="xn", name=f"wst{k}_{c}")
                        wengs[(k * 3 + c) % len(wengs)].dma_start(
                            wst, wv[:, k, c * E : (c + 1) * E]
                        )
                        nc.vector.tensor_copy(W[:, k, c * E : (c + 1) * E], wst)

            b3 = bvec[:].rearrange("(c m) -> c m", m=E)
            bb = constp.tile([128, 3, E], bf16)
            nc.gpsimd.dma_start(
                bb, bass.AP(tensor=b3.tensor, offset=b3.offset, ap=[[0, 128]] + list(b3.ap))
            )

            ident = constp.tile([128, 128], f32)
            make_identity(nc, ident)
            if variant < 36:
                identR = constp.tile([128, 128], f32r)
                make_identity(nc, identR)
            if 43 <= variant < 45:
                ones_b = constp.tile([128, 1], bf16)
                nc.gpsimd.memset(ones_b, 1.0)
                ones_f = constp.tile([128, 1], f32)
                nc.gpsimd.memset(ones_f, 1.0)

            def batch_body():
              for b in range(nb):
                v_w = 512 if 43 <= variant < 45 else 514
                Vx = bigp.tile([128, NE, v_w], bf16, tag="Vx")
                Qt = bigp.tile([128, ED, N], bf16, tag="Qt")
                Kt = bigp.tile([128, ED, N], bf16, tag="Kt")
                Vt = bigp.tile([128, ED, N], bf16, tag="Vt")
                PT = bigp.tile([128, NE, N], bf16, tag="PT")
                Acol = bigp.tile([128, ED, E], bf16, tag="Acol")
                AcolT = bigp.tile([128, ED, E], bf16, tag="AcolT")
                scol_ps = [
                    ps_sc.tile([128, E], f32, tag="scps", name=f"scol{b}_{i}")
                    for i in range(ED)
                ]
                if variant < 43 or variant >= 45:
                    # ones columns of Vx (positions 0 and 257) give the row
                    # sums of exp(S) as a byproduct of the PV accumulation
                    nc.gpsimd.memset(Vx[:, :, 0:1], 1.0)
                    nc.gpsimd.memset(Vx[:, :, 257:258], 1.0)

                # ---- phase A: load x, projections, col-S accumulation
                batched_t = variant >= 31
                pipe_xt = variant >= 33

                def load_xt(j):
                    """DMA chunk j of x and PE-transpose it into an xTc tile."""
                    xn = xnp.tile([128, E], f32, tag="xn")
                    nc.sync.dma_start(xn, x[b, j * 128 : (j + 1) * 128, :])
                    xTc = xtcp.tile([128, ED, 128], f32r, tag="xtc")
                    tg = ps_proj.tile([128, ED, 128], f32, tag="ps")
                    for k in range(ED):
                        nc.tensor.transpose(
                            tg[:, k, :], xn[:, k * 128 : (k + 1) * 128], ident
                        )
                    if variant >= 36:
                        nc.vector.tensor_copy(xTc, tg)
                    else:
                        nc.gpsimd.tensor_copy(xTc, tg)
                    return xTc

                if pipe_xt:
                    xTc_next = load_xt(0)
                for j in range(NE):
                    jsl = slice(j * 128, (j + 1) * 128)
                    if pipe_xt:
                        xTc = xTc_next
                        if j + 1 < NE:
                            xTc_next = load_xt(j + 1)
                    elif batched_t:
                        xTc = load_xt(j)
                    else:
                        xn = xnp.tile([128, E], f32, tag="xn")
                        nc.sync.dma_start(xn, x[b, jsl, :])
                        xTc = xtcp.tile([128, ED, 128], f32r, tag="xtc")
                        for k in range(ED):
                            tps = ps_proj.tile([128, 128], f32, tag="ps")
                            nc.tensor.transpose(
                                tps, xn[:, k * 128 : (k + 1) * 128], ident
                            )
                            nc.gpsimd.tensor_copy(xTc[:, k, :], tps)

                    qn = qknp.tile([128, E], f32r, tag="qn")
                    kn = qknp.tile([128, E], f32r, tag="kn")
                    if variant >= 32:
                        qnb = qknp.tile([128, E], bf16, tag="qnb")
                        knb = qknp.tile([128, E], bf16, tag="knb")
                    for dst, ci in ((qn, 0), (kn, 1)):
                        pp = ps_proj.tile([128, E], f32, tag="ps")
                        for k in range(ED):
                            nc.tensor.matmul(
                                pp,
                                xTc[:, k, :],
                                W[:, k, ci * E : (ci + 1) * E],
                                start=(k == 0),
                                stop=(k == ED - 1),
                            )
                        nc.vector.tensor_add(dst, pp, bb[:, ci, :])
                        if variant >= 33:
                            # bf16 copy (off the PSUM critical path: reads SBUF)
                            nc.gpsimd.tensor_copy((qnb, knb)[ci], dst)
                        elif variant >= 32:
                            # bf16 copy for the DMA-XBAR transpose into Qt/Kt
                            nc.gpsimd.tensor_add(
                                (qnb, knb)[ci], pp, bb[:, ci, :]
                            )
                    pp = ps_proj.tile([128, E], f32, tag="ps")
                    for k in range(ED):
                        nc.tensor.matmul(
                            pp,
                            xTc[:, k, :],
                            W[:, k, 2 * E : 3 * E],
                            start=(k == 0),
                            stop=(k == ED - 1),
                        )
                    veng = nc.vector if variant >= 36 else nc.gpsimd
                    if 43 <= variant < 45:
                        veng.tensor_add(Vx[:, j, :], pp, bb[:, 2, :])
                    else:
                        veng.tensor_add(Vx[:, j, 1:257], pp[:, 0:256], bb[:, 2, 0:256])
                        veng.tensor_add(Vx[:, j, 258:514], pp[:, 256:512], bb[:, 2, 256:512])

                    # col-attention S accumulation (f32r operands for precision)
                    for i in range(ED):
                        nc.tensor.matmul(
                            scol_ps[i],
                            qn[:, i * 128 : (i + 1) * 128],
                            kn,
                            start=(j == 0),
                            stop=(j == NE - 1),
                        )

                    # Qt/Kt via PE transpose (f32r 1.5 cyc/row), drained bf16
                    if variant >= 32:
                        qeng = nc.sync if variant >= 34 else nc.scalar
                        keng = nc.scalar
                        if variant >= 44:
                            qeng.dma_start_transpose(Qt[:, :, jsl], qnb)
                            keng.dma_start_transpose(Kt[:, :, jsl], knb)
                        elif variant != 41 or j == 0:
                            for h in range(2):
                                qeng.dma_start_transpose(
                                    Qt[:, 2 * h : 2 * h + 2, jsl],
                                    qnb[:, h * 256 : (h + 1) * 256],
                                )
                                keng.dma_start_transpose(
                                    Kt[:, 2 * h : 2 * h + 2, jsl],
                                    knb[:, h * 256 : (h + 1) * 256],
                                )
                    elif batched_t:
                        for src, dstT, deng in ((qn, Qt, 0), (kn, Kt, 1)):
                            tg = ps_proj.tile([128, ED, 128], f32r, tag="ps")
                            for k in range(ED):
                                nc.tensor.transpose(
                                    tg[:, k, :], src[:, k * 128 : (k + 1) * 128], identR
                                )
                            if deng == 0:
                                nc.scalar.activation(
                                    out=dstT[:, :, jsl], in_=tg, func=AF.Copy
                                )
                            else:
                                nc.vector.tensor_copy(dstT[:, :, jsl], tg)
                    else:
                        for src, dstT in ((qn, Qt), (kn, Kt)):
                            for k in range(ED):
                                tpr = ps_proj.tile([128, 128], f32r, tag="ps")
                                nc.tensor.transpose(
                                    tpr, src[:, k * 128 : (k + 1) * 128], identR
                                )
                                nc.scalar.activation(
                                    out=dstT[:, k, jsl], in_=tpr, func=AF.Copy
                                )

                    # Vt via DMA XBAR transpose
                    if 43 <= variant < 45:
                        nc.scalar.dma_start_transpose(Vt[:, :, jsl], Vx[:, j, :])
                    else:
                        for h in range(2):
                            nc.scalar.dma_start_transpose(
                                Vt[:, 2 * h : 2 * h + 2, jsl],
                                Vx[:, j, h * 257 + 1 : h * 257 + 257],
                            )

                # ---- phase A2: col softmax + transpose of A
                import contextlib
                a2ctx = tc.high_priority() if variant >= 37 else contextlib.nullcontext()
                with a2ctx:
                    cstat = statp.tile([128, 3 * ED], f32, tag="cstat")
                    for i in range(ED):
                        nm = cstat[:, 2 * ED + i : 2 * ED + i + 1]
                        nc.vector.reduce_max(nm, scol_ps[i], axis=AX.X, negate=True)
                        nc.vector.tensor_scalar_mul(nm, nm, SCALE)
                        nc.scalar.activation(
                            out=Acol[:, i, :],
                            in_=scol_ps[i],
                            func=AF.Exp,
                            scale=SCALE,
                            bias=nm,
                            accum_out=cstat[:, i : i + 1],
                        )
                    nc.vector.reciprocal(cstat[:, ED : 2 * ED], cstat[:, 0:ED])
                    for i in range(ED):
                        nc.vector.tensor_scalar_mul(
                            Acol[:, i, :], Acol[:, i, :], cstat[:, ED + i : ED + i + 1]
                        )
                        nc.scalar.dma_start_transpose(
                            AcolT[:, :, i * 128 : (i + 1) * 128], Acol[:, i, :]
                        )

                # ---- phase B1: S.T per m-chunk + exp straight into PT
                for mc in range(NE):
                    for q in range(ED):
                        sps = ps_s.tile([128, 512], f32, tag="s")
                        for k in range(ED):
                            nc.tensor.matmul(
                                sps,
                                Kt[:, k, mc * 128 : (mc + 1) * 128],
                                Qt[:, k, q * 512 : (q + 1) * 512],
                                start=(k == 0),
                                stop=(k == ED - 1),
                            )
                        nc.scalar.activation(
                            out=PT[:, mc, q * 512 : (q + 1) * 512],
                            in_=sps,
                            func=AF.Exp,
                            scale=SCALE,
                        )

                # ---- rowsum pass (v43+): rowsum_n = sum_m exp(S.T)[m, n]
                # via ones-stationary matmuls, then tiny on-chip transposes
                # to land 1/rowsum per token partition.
                if 43 <= variant < 45:
                    rsT = [
                        ps_sc.tile([128, 512], f32, tag="scps", name=f"rs{b}_{t}")
                        for t in range(2)
                    ]
                    for q4 in range(ED):
                        row = 32 * (q4 % 2)
                        nc_rs = rsT[q4 // 2]
                        for mc in range(NE):
                            nc.tensor.matmul(
                                nc_rs[row : row + 1, :],
                                ones_b,
                                PT[:, mc, q4 * 512 : (q4 + 1) * 512],
                                start=(mc == 0),
                                stop=(mc == NE - 1),
                            )
                    rss = [statp.tile([128, 512], f32, tag="rss", name=f"rss{b}_{t}", bufs=1) for t in range(2)]
                    for t in range(2):
                        for row in (0, 32):
                            nc.vector.tensor_copy(
                                rss[t][row : row + 1, :], rsT[t][row : row + 1, :]
                            )
                    rt_ps = ps_sc.tile([128, 16], f32, tag="scps", name=f"rt{b}")
                    for g in range(NE):
                        q4, jl = g // 4, g % 4
                        row = 32 * (q4 % 2)
                        nc.tensor.matmul(
                            rt_ps[:, g : g + 1],
                            rss[q4 // 2][row : row + 1, jl * 128 : (jl + 1) * 128],
                            ones_f[row : row + 1, :],
                            start=True,
                            stop=True,
                        )
                    rstatR = statp.tile([128, NE], f32, tag="rstatR", bufs=1)
                    nc.vector.reciprocal(rstatR, rt_ps)

                # ---- phase B2: PV + col output + merge
                for j in range(NE):
                    jsl = slice(j * 128, (j + 1) * 128)
                    if 43 <= variant < 45:
                        po0 = ps_sc.tile([128, E], f32, tag="scps")
                        for mc in range(NE):
                            nc.tensor.matmul(
                                po0,
                                PT[:, mc, jsl],
                                Vx[:, mc, :],
                                start=(mc == 0),
                                stop=(mc == NE - 1),
                            )
                    else:
                        po0 = ps_sc.tile([128, 257], f32, tag="scps")
                        po1 = ps_sc.tile([128, 257], f32, tag="scps")
                        for mc in range(NE):
                            nc.tensor.matmul(
                                po0,
                                PT[:, mc, jsl],
                                Vx[:, mc, 0:257],
                                start=(mc == 0),
                                stop=(mc == NE - 1),
                            )
                            if variant != 40:  # 40: timing probe, drops po1
                                nc.tensor.matmul(
                                    po1,
                                    PT[:, mc, jsl],
                                    Vx[:, mc, 257:514],
                                    start=(mc == 0),
                                    stop=(mc == NE - 1),
                                )
                        if variant == 40:
                            nc.tensor.matmul(
                                po1, PT[:, 0, jsl], Vx[:, 0, 257:514],
                                start=True, stop=True,
                            )
                    oc = ps_s.tile([128, E], f32, tag="s")
                    for c in range(ED):
                        nc.tensor.matmul(
                            oc,
                            Vt[:, c, jsl],
                            AcolT[:, c, :],
                            start=(c == 0),
                            stop=(c == ED - 1),
                        )
                    ot = outpp.tile([128, E], f32, tag="ot")
                    octmp = outpp.tile([128, E], f32, tag="octmp")
                    if 31 <= variant < 36:
                        nc.gpsimd.tensor_copy(octmp, oc)
                    else:
                        nc.scalar.activation(out=octmp, in_=oc, func=AF.Copy)
                    if 43 <= variant < 45:
                        nc.vector.scalar_tensor_tensor(
                            ot,
                            po0,
                            rstatR[:, j : j + 1],
                            octmp,
                            op0=mybir.AluOpType.mult,
                            op1=mybir.AluOpType.add,
                        )
                    else:
                        rstat = statp.tile([128, 1], f32, tag="rstat")
                        nc.vector.reciprocal(rstat, po0[:, 0:1])
                        nc.vector.scalar_tensor_tensor(
                            ot[:, 0:256],
                            po0[:, 1:257],
                            rstat,
                            octmp[:, 0:256],
                            op0=mybir.AluOpType.mult,
                            op1=mybir.AluOpType.add,
                        )
                        nc.vector.scalar_tensor_tensor(
                            ot[:, 256:512],
                            po1[:, 1:257],
                            rstat,
                            octmp[:, 256:512],
                            op0=mybir.AluOpType.mult,
                            op1=mybir.AluOpType.add,
                        )
                    nc.sync.dma_start(y[b, jsl, :], ot)

            if reps == 1:
                batch_body()
            else:
                with tc.For_i(0, reps, 1):
                    batch_body()

    nc.compile()
    _NC_CACHE[(nb, variant, reps)] = nc
    return nc


def make_in_maps(x, w_qkv, b_qkv):
    xs = np.ascontiguousarray(np.asarray(x, dtype=np.float32)).reshape(B, N, E)
    w = np.ascontiguousarray(np.asarray(w_qkv, dtype=np.float32))
    bq = np.ascontiguousarray(np.asarray(b_qkv, dtype=np.float32))
    return [
        {"x": np.ascontiguousarray(xs[c * NB : (c + 1) * NB]), "w": w, "b": bq}
        for c in range(NCORES)
    ]


BEST_VARIANT = 45


def kernel(x, w_qkv, b_qkv):
    from concourse.bass_utils import run_bass_kernel_spmd

    nc = build_nc(NB, BEST_VARIANT)
    in_maps = make_in_maps(x, w_qkv, b_qkv)
    res = run_bass_kernel_spmd(nc, in_maps, core_ids=list(range(NCORES)))
    out = np.empty((B, N, E), dtype=np.float32)
    for c in range(NCORES):
        out[c * NB : (c + 1) * NB] = res.results[c]["y"]
    return out

